# revision 13
# baseline (speedup 1.0000x reference)
"""Multi-head self-attention (Whisper-style, KV-cache tail write) on 8 trn2 cores.

Sharding: core c -> (batch b = c//2, head-group g = c%2).  Each core owns one
batch element and 8 of the 16 heads (a contiguous 1024-wide slice of D).
QKV/out projections are tensor-parallel over the head slice; the out-projection
partials of the two cores sharing a batch are summed on the host.

Device layout trick: the host passes x and the old K cache pre-transposed
(xT [D, QL], kcT_old [1024, 3072]) so every matmul contracts over the
partition dim with zero on-device transposes.  Attention runs in "sT" layout
(kpos on partitions): sT = kT_tile.T @ qT, exp on the scalar engine, wv and the
softmax denominators accumulate in PSUM over kpos tiles, and 1/denom is
broadcast back over partitions with a K=1 ones matmul.  All matmuls use the
float32r dtype (FP22 multiply, fp32 accumulate) which streams at 1 col/cycle.
"""

import os
import numpy as np

import concourse.bass as bass
import concourse.bacc as bacc
import concourse.mybir as mybir
from concourse.tile import TileContext
from concourse.bass_utils import run_bass_kernel_spmd

B, QL, KVL, D, H = 4, 1024, 2048 + 2048, 2048, 16
KVOLD = KVL - QL          # 3072
DH = D // H               # 128
DL = D // 2               # 1024 local width (8 heads per core)
HL = 8                    # local heads
NDT = D // 128            # 16 din tiles
NKT = KVL // 128          # 32 kpos tiles
NEWKT = QL // 128         # 8 new kpos tiles
SCALE = float(DH) ** -0.5
NEG = -1.0e12             # pre-scale mask add; SCALE*NEG still -> exp == 0

F32 = mybir.dt.float32
F32R = mybir.dt.float32r

N_CORES = 8


def build_nc(causal: bool) -> bass.Bass:
    nc = bacc.Bacc("TRN2", target_bir_lowering=False, debug=False)

    xT = nc.dram_tensor("xT", [D, QL], F32R, kind="ExternalInput").ap()
    wq = nc.dram_tensor("wq", [D, DL], F32R, kind="ExternalInput").ap()
    wk = nc.dram_tensor("wk", [D, DL], F32R, kind="ExternalInput").ap()
    wv_w = nc.dram_tensor("wv_w", [D, DL], F32R, kind="ExternalInput").ap()
    bq_d = nc.dram_tensor("bq_d", [1, DL], F32R, kind="ExternalInput").ap()
    bv_d = nc.dram_tensor("bv_d", [1, DL], F32R, kind="ExternalInput").ap()
    wo = nc.dram_tensor("wo", [DL, D], F32R, kind="ExternalInput").ap()
    kct_old = nc.dram_tensor("kct_old", [DL, KVOLD], F32R, kind="ExternalInput").ap()
    vc_old = nc.dram_tensor("vc_old", [KVOLD, DL], F32R, kind="ExternalInput").ap()
    tri = nc.dram_tensor("tri", [128, 128], F32, kind="ExternalInput").ap()
    ones_r_d = nc.dram_tensor("ones_r_d", [1, 512], F32R, kind="ExternalInput").ap()
    ones_c_d = nc.dram_tensor("ones_c_d", [128, 1], F32R, kind="ExternalInput").ap()

    ktn = nc.dram_tensor("ktn", [DL, QL], F32R, kind="ExternalOutput").ap()
    vn = nc.dram_tensor("vn", [QL, DL], F32R, kind="ExternalOutput").ap()
    outp = nc.dram_tensor("outp", [QL, D], F32, kind="ExternalOutput").ap()

    AF = mybir.ActivationFunctionType

    with TileContext(nc) as tc:
        with (
            tc.tile_pool(name="const", bufs=1) as cp,
            tc.tile_pool(name="qkv", bufs=8) as qkvp,
        ):
            ones_row = cp.tile([1, 512], F32R, name="ones_row", tag="ones_row")
            nc.sync.dma_start(ones_row[:], ones_r_d[:, :])
            ones_col = cp.tile([128, 1], F32R, name="ones_col", tag="ones_col")
            nc.sync.dma_start(ones_col[:], ones_c_d[:, :])
            tri_sb = cp.tile([128, 128], F32, name="tri_sb", tag="tri_sb")
            nc.sync.dma_start(tri_sb[:], tri[:, :])

            qT = [qkvp.tile([128, QL], F32R, name=f"qT{m}", tag="qT") for m in range(8)]
            kT = [qkvp.tile([128, QL], F32R, name=f"kT{m}", tag="kT") for m in range(8)]
            vv = [qkvp.tile([128, DL], F32R, name=f"vv{s}", tag="vv") for s in range(8)]

            # ---------------- phase A: projections ----------------
            with (
                tc.tile_pool(name="xp", bufs=16) as xp,
                tc.tile_pool(name="wp", bufs=24) as wp,
                tc.tile_pool(name="evA", bufs=4) as evp,
                tc.tile_pool(name="psA", bufs=4, space="PSUM") as pa,
            ):
                bq_sb = evp.tile([1, DL], F32R, name="bq_sb", tag="bq_sb", bufs=1)
                nc.sync.dma_start(bq_sb[:], bq_d[:, :])
                bv_sb = evp.tile([1, DL], F32R, name="bv_sb", tag="bv_sb", bufs=1)
                nc.sync.dma_start(bv_sb[:], bv_d[:, :])
                xT_t = []
                for dt in range(NDT):
                    t = xp.tile([128, QL], F32R, name=f"xTt{dt}", tag="xT")
                    nc.sync.dma_start(t[:], xT[dt * 128:(dt + 1) * 128, :])
                    xT_t.append(t)

                # qT and kT: out[dout_tile, seq] = W.T @ xT  (lhsT = W)
                for w_ap, bias_sb, out_arr, out_dram in (
                    (wq, bq_sb, qT, None),
                    (wk, None, kT, ktn),
                ):
                    for wc in range(4):
                        wt = []
                        for dt in range(NDT):
                            t = wp.tile([128, 256], F32R, name=f"w{wc}_{dt}", tag="w")
                            nc.sync.dma_start(
                                t[:], w_ap[dt * 128:(dt + 1) * 128,
                                           wc * 256:(wc + 1) * 256])
                            wt.append(t)
                        for mi in range(2):
                            m = wc * 2 + mi
                            for qh in range(2):
                                ps = pa.tile([128, 512], F32, name=f"psA{m}{qh}",
                                             tag="psA")
                                for dt in range(NDT):
                                    nc.tensor.matmul(
                                        ps[:],
                                        wt[dt][:, mi * 128:(mi + 1) * 128],
                                        xT_t[dt][:, qh * 512:(qh + 1) * 512],
                                        start=(dt == 0),
                                        stop=(dt == NDT - 1 and bias_sb is None),
                                    )
                                if bias_sb is not None:
                                    nc.tensor.matmul(
                                        ps[:],
                                        bias_sb[:, m * 128:(m + 1) * 128],
                                        ones_row[:, :512],
                                        start=False, stop=True,
                                    )
                                nc.vector.tensor_copy(
                                    out_arr[m][:, qh * 512:(qh + 1) * 512], ps[:])
                    if out_dram is not None:
                        for m in range(8):
                            nc.sync.dma_start(
                                out_dram[m * 128:(m + 1) * 128, :], out_arr[m][:])

                # v: out[seq_tile, dout] = xT.T @ Wv  (lhsT = xT)
                for wc in range(4):
                    wt = []
                    for dt in range(NDT):
                        t = wp.tile([128, 256], F32R, name=f"wv{wc}_{dt}", tag="w")
                        nc.sync.dma_start(
                            t[:], wv_w[dt * 128:(dt + 1) * 128,
                                       wc * 256:(wc + 1) * 256])
                        wt.append(t)
                    for s in range(8):
                        ps = pa.tile([128, 256], F32, name=f"psV{s}", tag="psA")
                        for dt in range(NDT):
                            nc.tensor.matmul(
                                ps[:],
                                xT_t[dt][:, s * 128:(s + 1) * 128],
                                wt[dt][:],
                                start=(dt == 0), stop=False,
                            )
                        nc.tensor.matmul(
                            ps[:],
                            ones_row[:, :128],
                            bv_sb[:, wc * 256:(wc + 1) * 256],
                            start=False, stop=True,
                        )
                        nc.vector.tensor_copy(
                            vv[s][:, wc * 256:(wc + 1) * 256], ps[:])
                for s in range(8):
                    nc.sync.dma_start(vn[s * 128:(s + 1) * 128, :], vv[s][:])

            # ---------------- phase B: attention ----------------
            with tc.tile_pool(name="wvTp", bufs=8) as wvtp:
                wvT = [wvtp.tile([128, QL], F32R, name=f"wvT{h}", tag="wvT")
                       for h in range(HL)]
                with (
                    tc.tile_pool(name="kchp", bufs=2) as kchp,
                    tc.tile_pool(name="vchp", bufs=2) as vchp,
                    tc.tile_pool(name="expp", bufs=2) as xpp,
                    tc.tile_pool(name="rbp", bufs=1) as rbp,
                    tc.tile_pool(name="dsp", bufs=3) as dsp,
                    tc.tile_pool(name="psB", bufs=1, space="PSUM") as pb,
                ):
                    # vc_old rows for head h as [128, (t c)] SBUF tile: partition
                    # p, free (t, c) <- HBM row t*128+p, col h*128+c
                    vco_r = vc_old.rearrange("(t p) c -> p t c", p=128)

                    for h in range(HL):
                        kch = kchp.tile([128, KVOLD], F32R, name=f"kch{h}", tag="kch")
                        nc.sync.dma_start(
                            kch[:], kct_old[h * 128:(h + 1) * 128, :])
                        vch = vchp.tile([128, KVOLD], F32R, name=f"vch{h}", tag="vch")
                        nc.sync.dma_start(
                            vch[:].rearrange("p (t c) -> p t c", c=128),
                            vco_r[:, :, h * 128:(h + 1) * 128])

                        wv_ps = pb.tile([128, QL], F32, name=f"wvps{h}", tag="wv",
                                        bufs=1)
                        den_lo = pb.tile([1, 512], F32, name=f"denlo{h}",
                                         tag="den_lo", bufs=1)
                        den_hi = pb.tile([1, 512], F32, name=f"denhi{h}",
                                         tag="den_hi", bufs=1)

                        for kt in range(NKT):
                            j = kt - (NKT - NEWKT)  # >= 0 in the new-cache region
                            qs = 128 * j if (causal and j > 0) else 0
                            if j >= 0:
                                k_l = kT[h][:, j * 128:(j + 1) * 128]
                                v_l = vv[j][:, h * 128:(h + 1) * 128]
                            else:
                                k_l = kch[:, kt * 128:(kt + 1) * 128]
                                v_l = vch[:, kt * 128:(kt + 1) * 128]

                            segs = ([(qs, 512), (512, QL)] if qs < 512
                                    else [(qs, QL)])
                            st = pb.tile([128, QL], F32, name=f"st{h}_{kt}",
                                         tag="sT", bufs=2)
                            for a, e in segs:
                                nc.tensor.matmul(
                                    st[:, a:e], k_l, qT[h][:, a:e],
                                    start=True, stop=True)
                            if causal and j >= 0:
                                nc.vector.tensor_add(
                                    st[:, qs:qs + 128], st[:, qs:qs + 128],
                                    tri_sb[:])
                            et = xpp.tile([128, QL], F32R, name=f"et{h}_{kt}",
                                          tag="expp")
                            nc.scalar.activation(
                                et[:, qs:QL], st[:, qs:QL], AF.Exp, scale=SCALE)

                            first = (kt == 0)
                            # last kt writing the low half ([0:512)) vs high half
                            last_lo = (NKT - NEWKT + 3) if causal else (NKT - 1)
                            for a, e in segs:
                                lo = a < 512
                                stop = (kt == (last_lo if lo else NKT - 1))
                                nc.tensor.matmul(
                                    wv_ps[:, a:e], v_l, et[:, a:e],
                                    start=first, stop=stop,
                                    skip_group_check=True)
                                if lo:
                                    d_out = den_lo[0:1, a:512]
                                else:
                                    d_out = den_hi[0:1, a - 512:e - 512]
                                nc.tensor.matmul(
                                    d_out, ones_col[:, :], et[:, a:e],
                                    start=first, stop=stop,
                                    skip_group_check=True)

                        # evacuate unnormalized wv and the denominators
                        nc.vector.tensor_copy(wvT[h][:], wv_ps[:])
                        den_sb = dsp.tile([1, QL], F32, name=f"densb{h}", tag="ds")
                        nc.vector.tensor_copy(den_sb[:, 0:512], den_lo[0:1, :])
                        nc.vector.tensor_copy(den_sb[:, 512:QL], den_hi[0:1, :])
                        ln_sb = dsp.tile([1, QL], F32, name=f"lnsb{h}", tag="ds")
                        nc.scalar.activation(ln_sb[:], den_sb[:], AF.Ln)
                        rec_sb = dsp.tile([1, QL], F32R, name=f"recsb{h}",
                                          tag="ds")
                        nc.scalar.activation(rec_sb[:], ln_sb[:], AF.Exp,
                                             scale=-1.0)
                        rb = rbp.tile([128, QL], F32R, name=f"rb{h}", tag="rb")
                        nc.gpsimd.partition_broadcast(rb[:], rec_sb[:])
                        nc.vector.tensor_mul(wvT[h][:], wvT[h][:], rb[:])

                # ---------------- phase C: out projection ----------------
                with (
                    tc.tile_pool(name="wop", bufs=16) as wop,
                    tc.tile_pool(name="ocp", bufs=4) as ocp,
                    tc.tile_pool(name="psC", bufs=4, space="PSUM") as pc,
                ):
                    for n in range(4):
                        wot = []
                        for dt in range(8):
                            t = wop.tile([128, 512], F32R, name=f"wo{n}_{dt}",
                                         tag="wo")
                            nc.sync.dma_start(
                                t[:], wo[dt * 128:(dt + 1) * 128,
                                         n * 512:(n + 1) * 512])
                            wot.append(t)
                        for m in range(8):
                            ps = pc.tile([128, 512], F32, name=f"psC{n}{m}",
                                         tag="psC")
                            for dt in range(8):
                                nc.tensor.matmul(
                                    ps[:],
                                    wvT[dt][:, m * 128:(m + 1) * 128],
                                    wot[dt][:],
                                    start=(dt == 0), stop=(dt == 7))
                            ot = ocp.tile([128, 512], F32, name=f"oc{n}{m}",
                                          tag="oc")
                            nc.vector.tensor_copy(ot[:], ps[:])
                            nc.sync.dma_start(
                                outp[m * 128:(m + 1) * 128,
                                     n * 512:(n + 1) * 512], ot[:])
    nc.finalize()
    return nc


_NC_CACHE: dict = {}


def _get_nc(causal: bool) -> bass.Bass:
    if causal not in _NC_CACHE:
        _NC_CACHE[causal] = build_nc(causal)
    return _NC_CACHE[causal]


def _check_mask(mask: np.ndarray) -> bool:
    """True -> the standard causal mask; False -> all-zeros (full attention)."""
    if not np.any(mask):
        return False
    i = np.arange(QL)[:, None]
    j = np.arange(KVL)[None, :]
    causal = np.where(j > (KVL - QL) + i, float(np.min(mask)), 0.0)
    if np.array_equal(mask, causal.astype(mask.dtype)) and np.min(mask) < -1e6:
        return True
    raise ValueError("kernel only supports the causal or empty mask")


def _run(inputs: dict, trace: bool = False):
    x = np.asarray(inputs["x"], dtype=np.float32)
    k_cache = np.asarray(inputs["k_cache"], dtype=np.float32)
    v_cache = np.asarray(inputs["v_cache"], dtype=np.float32)
    mask = np.asarray(inputs["mask"], dtype=np.float32)
    Wq = np.asarray(inputs["Wq"], dtype=np.float32)
    bq = np.asarray(inputs["bq"], dtype=np.float32)
    Wk = np.asarray(inputs["Wk"], dtype=np.float32)
    Wv = np.asarray(inputs["Wv"], dtype=np.float32)
    bv = np.asarray(inputs["bv"], dtype=np.float32)
    Wo = np.asarray(inputs["Wo"], dtype=np.float32)
    bo = np.asarray(inputs["bo"], dtype=np.float32)

    causal = _check_mask(mask)
    nc = _get_nc(causal)

    tri_np = np.where(
        np.arange(128)[None, :] < np.arange(128)[:, None], NEG, 0.0
    ).astype(np.float32)

    in_maps = []
    for c in range(N_CORES):
        b, g = c // 2, c % 2
        cs = slice(g * DL, (g + 1) * DL)
        in_maps.append({
            "xT": np.ascontiguousarray(x[b].T),
            "wq": np.ascontiguousarray(Wq[:, cs]),
            "wk": np.ascontiguousarray(Wk[:, cs]),
            "wv_w": np.ascontiguousarray(Wv[:, cs]),
            "bq_d": np.ascontiguousarray(bq[cs][None, :]),
            "bv_d": np.ascontiguousarray(bv[cs][None, :]),
            "wo": np.ascontiguousarray(Wo[cs, :]),
            "kct_old": np.ascontiguousarray(k_cache[b, :KVOLD, cs].T),
            "vc_old": np.ascontiguousarray(v_cache[b, :KVOLD, cs]),
            "tri": tri_np,
            "ones_r_d": np.ones((1, 512), np.float32),
            "ones_c_d": np.ones((128, 1), np.float32),
        })

    bkr = run_bass_kernel_spmd(nc, in_maps, list(range(N_CORES)), trace=trace)
    res = bkr.results

    kc = k_cache.copy()
    vc = v_cache.copy()
    out = np.empty((B, QL, D), dtype=np.float32)
    for c in range(N_CORES):
        b, g = c // 2, c % 2
        cs = slice(g * DL, (g + 1) * DL)
        kc[b, KVOLD:, cs] = res[c]["ktn"].T
        vc[b, KVOLD:, cs] = res[c]["vn"]
    for b in range(B):
        out[b] = res[2 * b]["outp"] + res[2 * b + 1]["outp"] + bo

    return (out, kc, vc), bkr


def kernel(**inputs):
    (out, kc, vc), _ = _run(inputs, trace=bool(os.environ.get("BASS_TRACE")))
    return out, kc, vc


# revision 14
# speedup vs baseline: 1.0198x; 1.0198x over previous
"""Multi-head self-attention (Whisper-style, KV-cache tail write) on 8 trn2 cores.

Sharding: core c -> (batch b = c//2, head-group g = c%2).  Each core owns one
batch element and 8 of the 16 heads (a contiguous 1024-wide slice of D).
QKV/out projections are tensor-parallel over the head slice; the out-projection
partials of the two cores sharing a batch are summed on the host.

Device layout trick: the host passes x and the old K cache pre-transposed
(xT [D, QL], kcT_old [1024, 3072]) so every matmul contracts over the
partition dim with zero on-device transposes.  Attention runs in "sT" layout
(kpos on partitions): sT = kT_tile.T @ qT, exp on the scalar engine, wv and the
softmax denominators accumulate in PSUM over kpos tiles, and 1/denom is
broadcast back over partitions with a K=1 ones matmul.  All matmuls use the
float32r dtype (FP22 multiply, fp32 accumulate) which streams at 1 col/cycle.
"""

import os
import numpy as np

import concourse.bass as bass
import concourse.bacc as bacc
import concourse.mybir as mybir
from concourse.tile import TileContext
from concourse.bass_utils import run_bass_kernel_spmd

B, QL, KVL, D, H = 4, 1024, 2048 + 2048, 2048, 16
KVOLD = KVL - QL          # 3072
DH = D // H               # 128
DL = D // 2               # 1024 local width (8 heads per core)
HL = 8                    # local heads
NDT = D // 128            # 16 din tiles
NKT = KVL // 128          # 32 kpos tiles
NEWKT = QL // 128         # 8 new kpos tiles
SCALE = float(DH) ** -0.5
NEG = -1.0e12             # pre-scale mask add; SCALE*NEG still -> exp == 0

F32 = mybir.dt.float32
F32R = mybir.dt.float32r

N_CORES = 8


def build_nc(causal: bool) -> bass.Bass:
    nc = bacc.Bacc("TRN2", target_bir_lowering=False, debug=False)

    xT = nc.dram_tensor("xT", [D, QL], F32R, kind="ExternalInput").ap()
    wq = nc.dram_tensor("wq", [D, DL], F32R, kind="ExternalInput").ap()
    wk = nc.dram_tensor("wk", [D, DL], F32R, kind="ExternalInput").ap()
    wv_w = nc.dram_tensor("wv_w", [D, DL], F32R, kind="ExternalInput").ap()
    bq_d = nc.dram_tensor("bq_d", [1, DL], F32R, kind="ExternalInput").ap()
    bv_d = nc.dram_tensor("bv_d", [1, DL], F32R, kind="ExternalInput").ap()
    wo = nc.dram_tensor("wo", [DL, D], F32R, kind="ExternalInput").ap()
    kct_old = nc.dram_tensor("kct_old", [DL, KVOLD], F32R, kind="ExternalInput").ap()
    vc_old = nc.dram_tensor("vc_old", [KVOLD, DL], F32R, kind="ExternalInput").ap()
    tri = nc.dram_tensor("tri", [128, 128], F32, kind="ExternalInput").ap()
    ones_r_d = nc.dram_tensor("ones_r_d", [1, 512], F32R, kind="ExternalInput").ap()
    ones_c_d = nc.dram_tensor("ones_c_d", [128, 1], F32R, kind="ExternalInput").ap()

    ktn = nc.dram_tensor("ktn", [DL, QL], F32R, kind="ExternalOutput").ap()
    vn = nc.dram_tensor("vn", [QL, DL], F32R, kind="ExternalOutput").ap()
    outp = nc.dram_tensor("outp", [QL, D], F32, kind="ExternalOutput").ap()

    AF = mybir.ActivationFunctionType

    with TileContext(nc) as tc:
        with (
            tc.tile_pool(name="const", bufs=1) as cp,
            tc.tile_pool(name="qkv", bufs=8) as qkvp,
        ):
            ones_row = cp.tile([1, 512], F32R, name="ones_row", tag="ones_row")
            nc.sync.dma_start(ones_row[:], ones_r_d[:, :])
            ones_col = cp.tile([128, 1], F32R, name="ones_col", tag="ones_col")
            nc.sync.dma_start(ones_col[:], ones_c_d[:, :])
            tri_sb = cp.tile([128, 128], F32, name="tri_sb", tag="tri_sb")
            nc.sync.dma_start(tri_sb[:], tri[:, :])

            qT = [qkvp.tile([128, QL], F32R, name=f"qT{m}", tag="qT") for m in range(8)]
            kT = [qkvp.tile([128, QL], F32R, name=f"kT{m}", tag="kT") for m in range(8)]
            vv = [qkvp.tile([128, DL], F32R, name=f"vv{s}", tag="vv") for s in range(8)]

            # ---------------- phase A: projections ----------------
            with (
                tc.tile_pool(name="xp", bufs=16) as xp,
                tc.tile_pool(name="wp", bufs=24) as wp,
                tc.tile_pool(name="evA", bufs=4) as evp,
                tc.tile_pool(name="psA", bufs=4, space="PSUM") as pa,
            ):
                bq_sb = evp.tile([1, DL], F32R, name="bq_sb", tag="bq_sb", bufs=1)
                nc.sync.dma_start(bq_sb[:], bq_d[:, :])
                bv_sb = evp.tile([1, DL], F32R, name="bv_sb", tag="bv_sb", bufs=1)
                nc.sync.dma_start(bv_sb[:], bv_d[:, :])
                xT_t = []
                for dt in range(NDT):
                    t = xp.tile([128, QL], F32R, name=f"xTt{dt}", tag="xT")
                    nc.sync.dma_start(t[:], xT[dt * 128:(dt + 1) * 128, :])
                    xT_t.append(t)

                # qT and kT: out[dout_tile, seq] = W.T @ xT  (lhsT = W)
                for w_ap, bias_sb, out_arr, out_dram in (
                    (wq, bq_sb, qT, None),
                    (wk, None, kT, ktn),
                ):
                    for wc in range(4):
                        wt = []
                        for dt in range(NDT):
                            t = wp.tile([128, 256], F32R, name=f"w{wc}_{dt}", tag="w")
                            nc.sync.dma_start(
                                t[:], w_ap[dt * 128:(dt + 1) * 128,
                                           wc * 256:(wc + 1) * 256])
                            wt.append(t)
                        for mi in range(2):
                            m = wc * 2 + mi
                            for qh in range(2):
                                ps = pa.tile([128, 512], F32, name=f"psA{m}{qh}",
                                             tag="psA")
                                for dt in range(NDT):
                                    nc.tensor.matmul(
                                        ps[:],
                                        wt[dt][:, mi * 128:(mi + 1) * 128],
                                        xT_t[dt][:, qh * 512:(qh + 1) * 512],
                                        start=(dt == 0),
                                        stop=(dt == NDT - 1 and bias_sb is None),
                                    )
                                if bias_sb is not None:
                                    nc.tensor.matmul(
                                        ps[:],
                                        bias_sb[:, m * 128:(m + 1) * 128],
                                        ones_row[:, :512],
                                        start=False, stop=True,
                                    )
                                nc.vector.tensor_copy(
                                    out_arr[m][:, qh * 512:(qh + 1) * 512], ps[:])
                    if out_dram is not None:
                        for m in range(8):
                            nc.sync.dma_start(
                                out_dram[m * 128:(m + 1) * 128, :], out_arr[m][:])

                # v: out[seq_tile, dout] = xT.T @ Wv  (lhsT = xT)
                for wc in range(4):
                    wt = []
                    for dt in range(NDT):
                        t = wp.tile([128, 256], F32R, name=f"wv{wc}_{dt}", tag="w")
                        nc.sync.dma_start(
                            t[:], wv_w[dt * 128:(dt + 1) * 128,
                                       wc * 256:(wc + 1) * 256])
                        wt.append(t)
                    for s in range(8):
                        ps = pa.tile([128, 256], F32, name=f"psV{s}", tag="psA")
                        for dt in range(NDT):
                            nc.tensor.matmul(
                                ps[:],
                                xT_t[dt][:, s * 128:(s + 1) * 128],
                                wt[dt][:],
                                start=(dt == 0), stop=False,
                            )
                        nc.tensor.matmul(
                            ps[:],
                            ones_row[:, :128],
                            bv_sb[:, wc * 256:(wc + 1) * 256],
                            start=False, stop=True,
                        )
                        nc.vector.tensor_copy(
                            vv[s][:, wc * 256:(wc + 1) * 256], ps[:])
                for s in range(8):
                    nc.sync.dma_start(vn[s * 128:(s + 1) * 128, :], vv[s][:])

            # ---------------- phase B: attention ----------------
            with tc.tile_pool(name="wvTp", bufs=8) as wvtp:
                wvT = [wvtp.tile([128, QL], F32R, name=f"wvT{h}", tag="wvT")
                       for h in range(HL)]
                with (
                    tc.tile_pool(name="kchp", bufs=2) as kchp,
                    tc.tile_pool(name="vchp", bufs=2) as vchp,
                    tc.tile_pool(name="expp", bufs=3) as xpp,
                    tc.tile_pool(name="dsp", bufs=3) as dsp,
                    tc.tile_pool(name="psB", bufs=1, space="PSUM") as pb,
                ):
                    # vc_old rows for head h as [128, (t c)] SBUF tile: partition
                    # p, free (t, c) <- HBM row t*128+p, col h*128+c
                    vco_r = vc_old.rearrange("(t p) c -> p t c", p=128)

                    for h in range(HL):
                        kch = kchp.tile([128, KVOLD], F32R, name=f"kch{h}", tag="kch")
                        vch = vchp.tile([128, KVOLD], F32R, name=f"vch{h}", tag="vch")
                        for q4 in range(4):
                            cw = KVOLD // 4
                            sl = slice(q4 * cw, (q4 + 1) * cw)
                            nc.sync.dma_start(
                                kch[:, sl], kct_old[h * 128:(h + 1) * 128, sl])
                            nc.sync.dma_start(
                                vch[:, sl].rearrange("p (t c) -> p t c", c=128),
                                vco_r[:, q4 * (cw // 128):(q4 + 1) * (cw // 128),
                                      h * 128:(h + 1) * 128])

                        wv_ps = pb.tile([128, QL], F32, name=f"wvps{h}", tag="wv",
                                        bufs=1)
                        den_lo = pb.tile([1, 512], F32, name=f"denlo{h}",
                                         tag="den_lo", bufs=1)
                        den_hi = pb.tile([1, 512], F32, name=f"denhi{h}",
                                         tag="den_hi", bufs=1)

                        for kt in range(NKT):
                            j = kt - (NKT - NEWKT)  # >= 0 in the new-cache region
                            qs = 128 * j if (causal and j > 0) else 0
                            if j >= 0:
                                k_l = kT[h][:, j * 128:(j + 1) * 128]
                                v_l = vv[j][:, h * 128:(h + 1) * 128]
                            else:
                                k_l = kch[:, kt * 128:(kt + 1) * 128]
                                v_l = vch[:, kt * 128:(kt + 1) * 128]

                            segs = ([(qs, 512), (512, QL)] if qs < 512
                                    else [(qs, QL)])
                            st = pb.tile([128, QL], F32, name=f"st{h}_{kt}",
                                         tag="sT", bufs=2)
                            for a, e in segs:
                                nc.tensor.matmul(
                                    st[:, a:e], k_l, qT[h][:, a:e],
                                    start=True, stop=True)
                            if causal and j >= 0:
                                nc.vector.tensor_add(
                                    st[:, qs:qs + 128], st[:, qs:qs + 128],
                                    tri_sb[:])
                            et = xpp.tile([128, QL], F32R, name=f"et{h}_{kt}",
                                          tag="expp")
                            nc.scalar.activation(
                                et[:, qs:QL], st[:, qs:QL], AF.Exp, scale=SCALE)

                            first = (kt == 0)
                            # last kt writing the low half ([0:512)) vs high half
                            last_lo = (NKT - NEWKT + 3) if causal else (NKT - 1)
                            for a, e in segs:
                                lo = a < 512
                                stop = (kt == (last_lo if lo else NKT - 1))
                                nc.tensor.matmul(
                                    wv_ps[:, a:e], v_l, et[:, a:e],
                                    start=first, stop=stop,
                                    skip_group_check=True)
                                if lo:
                                    d_out = den_lo[0:1, a:512]
                                else:
                                    d_out = den_hi[0:1, a - 512:e - 512]
                                nc.tensor.matmul(
                                    d_out, ones_col[:, :], et[:, a:e],
                                    start=first, stop=stop,
                                    skip_group_check=True)

                        # evacuate unnormalized wv and the denominators
                        nc.vector.tensor_copy(wvT[h][:], wv_ps[:])
                        den_sb = dsp.tile([1, QL], F32, name=f"densb{h}", tag="ds")
                        nc.vector.tensor_copy(den_sb[:, 0:512], den_lo[0:1, :])
                        nc.vector.tensor_copy(den_sb[:, 512:QL], den_hi[0:1, :])
                        ln_sb = dsp.tile([1, QL], F32, name=f"lnsb{h}", tag="ds")
                        nc.scalar.activation(ln_sb[:], den_sb[:], AF.Ln)
                        rec_sb = dsp.tile([1, QL], F32R, name=f"recsb{h}",
                                          tag="ds")
                        nc.scalar.activation(rec_sb[:], ln_sb[:], AF.Exp,
                                             scale=-1.0)
                        aux = pb.tile([128, QL], F32, name=f"aux{h}",
                                      tag="sT", bufs=2)
                        for a in (0, 512):
                            nc.tensor.matmul(
                                aux[:, a:a + 512], ones_row[:, :128],
                                rec_sb[:, a:a + 512], start=True, stop=True)
                        nc.vector.tensor_mul(wvT[h][:], wvT[h][:], aux[:])

                # ---------------- phase C: out projection ----------------
                with (
                    tc.tile_pool(name="wop", bufs=16) as wop,
                    tc.tile_pool(name="ocp", bufs=4) as ocp,
                    tc.tile_pool(name="psC", bufs=4, space="PSUM") as pc,
                ):
                    for n in range(4):
                        wot = []
                        for dt in range(8):
                            t = wop.tile([128, 512], F32R, name=f"wo{n}_{dt}",
                                         tag="wo")
                            nc.sync.dma_start(
                                t[:], wo[dt * 128:(dt + 1) * 128,
                                         n * 512:(n + 1) * 512])
                            wot.append(t)
                        for m in range(8):
                            ps = pc.tile([128, 512], F32, name=f"psC{n}{m}",
                                         tag="psC")
                            for dt in range(8):
                                nc.tensor.matmul(
                                    ps[:],
                                    wvT[dt][:, m * 128:(m + 1) * 128],
                                    wot[dt][:],
                                    start=(dt == 0), stop=(dt == 7))
                            ot = ocp.tile([128, 512], F32, name=f"oc{n}{m}",
                                          tag="oc")
                            nc.vector.tensor_copy(ot[:], ps[:])
                            nc.sync.dma_start(
                                outp[m * 128:(m + 1) * 128,
                                     n * 512:(n + 1) * 512], ot[:])
    nc.finalize()
    return nc


_NC_CACHE: dict = {}


def _get_nc(causal: bool) -> bass.Bass:
    if causal not in _NC_CACHE:
        _NC_CACHE[causal] = build_nc(causal)
    return _NC_CACHE[causal]


def _check_mask(mask: np.ndarray) -> bool:
    """True -> the standard causal mask; False -> all-zeros (full attention)."""
    if not np.any(mask):
        return False
    i = np.arange(QL)[:, None]
    j = np.arange(KVL)[None, :]
    causal = np.where(j > (KVL - QL) + i, float(np.min(mask)), 0.0)
    if np.array_equal(mask, causal.astype(mask.dtype)) and np.min(mask) < -1e6:
        return True
    raise ValueError("kernel only supports the causal or empty mask")


def _run(inputs: dict, trace: bool = False):
    x = np.asarray(inputs["x"], dtype=np.float32)
    k_cache = np.asarray(inputs["k_cache"], dtype=np.float32)
    v_cache = np.asarray(inputs["v_cache"], dtype=np.float32)
    mask = np.asarray(inputs["mask"], dtype=np.float32)
    Wq = np.asarray(inputs["Wq"], dtype=np.float32)
    bq = np.asarray(inputs["bq"], dtype=np.float32)
    Wk = np.asarray(inputs["Wk"], dtype=np.float32)
    Wv = np.asarray(inputs["Wv"], dtype=np.float32)
    bv = np.asarray(inputs["bv"], dtype=np.float32)
    Wo = np.asarray(inputs["Wo"], dtype=np.float32)
    bo = np.asarray(inputs["bo"], dtype=np.float32)

    causal = _check_mask(mask)
    nc = _get_nc(causal)

    tri_np = np.where(
        np.arange(128)[None, :] < np.arange(128)[:, None], NEG, 0.0
    ).astype(np.float32)

    in_maps = []
    for c in range(N_CORES):
        b, g = c // 2, c % 2
        cs = slice(g * DL, (g + 1) * DL)
        in_maps.append({
            "xT": np.ascontiguousarray(x[b].T),
            "wq": np.ascontiguousarray(Wq[:, cs]),
            "wk": np.ascontiguousarray(Wk[:, cs]),
            "wv_w": np.ascontiguousarray(Wv[:, cs]),
            "bq_d": np.ascontiguousarray(bq[cs][None, :]),
            "bv_d": np.ascontiguousarray(bv[cs][None, :]),
            "wo": np.ascontiguousarray(Wo[cs, :]),
            "kct_old": np.ascontiguousarray(k_cache[b, :KVOLD, cs].T),
            "vc_old": np.ascontiguousarray(v_cache[b, :KVOLD, cs]),
            "tri": tri_np,
            "ones_r_d": np.ones((1, 512), np.float32),
            "ones_c_d": np.ones((128, 1), np.float32),
        })

    bkr = run_bass_kernel_spmd(nc, in_maps, list(range(N_CORES)), trace=trace)
    res = bkr.results

    kc = k_cache.copy()
    vc = v_cache.copy()
    out = np.empty((B, QL, D), dtype=np.float32)
    for c in range(N_CORES):
        b, g = c // 2, c % 2
        cs = slice(g * DL, (g + 1) * DL)
        kc[b, KVOLD:, cs] = res[c]["ktn"].T
        vc[b, KVOLD:, cs] = res[c]["vn"]
    for b in range(B):
        out[b] = res[2 * b]["outp"] + res[2 * b + 1]["outp"] + bo

    return (out, kc, vc), bkr


def kernel(**inputs):
    (out, kc, vc), _ = _run(inputs, trace=bool(os.environ.get("BASS_TRACE")))
    return out, kc, vc


# revision 15
# speedup vs baseline: 1.1031x; 1.0817x over previous
"""Multi-head self-attention (Whisper-style, KV-cache tail write) on 8 trn2 cores.

Sharding: core c -> (batch b = c//2, head-group g = c%2).  Each core owns one
batch element and 8 of the 16 heads (a contiguous 1024-wide slice of D).
QKV/out projections are tensor-parallel over the head slice; the out-projection
partials of the two cores sharing a batch are summed on the host.

Device layout trick: the host passes x and the old K cache pre-transposed
(xT [D, QL], kcT_old [1024, 3072]) so every matmul contracts over the
partition dim with zero on-device transposes.  Attention runs in "sT" layout
(kpos on partitions): sT = kT_tile.T @ qT, exp on the scalar engine, wv and the
softmax denominators accumulate in PSUM over kpos tiles, and 1/denom is
broadcast back over partitions with a K=1 ones matmul.  All matmuls use the
float32r dtype (FP22 multiply, fp32 accumulate) which streams at 1 col/cycle.
"""

import os
import numpy as np

import concourse.bass as bass
import concourse.bacc as bacc
import concourse.mybir as mybir
from concourse.tile import TileContext
from concourse.bass_utils import run_bass_kernel_spmd

B, QL, KVL, D, H = 4, 1024, 2048 + 2048, 2048, 16
KVOLD = KVL - QL          # 3072
DH = D // H               # 128
DL = D // 2               # 1024 local width (8 heads per core)
HL = 8                    # local heads
NDT = D // 128            # 16 din tiles
NKT = KVL // 128          # 32 kpos tiles
NEWKT = QL // 128         # 8 new kpos tiles
SCALE = float(DH) ** -0.5
NEG = -1.0e12             # pre-scale mask add; SCALE*NEG still -> exp == 0

F32 = mybir.dt.float32
F32R = mybir.dt.float32r

N_CORES = 8


def build_nc(causal: bool) -> bass.Bass:
    nc = bacc.Bacc("TRN2", target_bir_lowering=False, debug=False)

    xT = nc.dram_tensor("xT", [D, QL], F32R, kind="ExternalInput").ap()
    wq = nc.dram_tensor("wq", [D, DL], F32R, kind="ExternalInput").ap()
    wk = nc.dram_tensor("wk", [D, DL], F32R, kind="ExternalInput").ap()
    wv_w = nc.dram_tensor("wv_w", [D, DL], F32R, kind="ExternalInput").ap()
    bq_d = nc.dram_tensor("bq_d", [1, DL], F32R, kind="ExternalInput").ap()
    bv_d = nc.dram_tensor("bv_d", [1, DL], F32R, kind="ExternalInput").ap()
    wo = nc.dram_tensor("wo", [DL, D], F32R, kind="ExternalInput").ap()
    kct_old = nc.dram_tensor("kct_old", [DL, KVOLD], F32R, kind="ExternalInput").ap()
    vc_old = nc.dram_tensor("vc_old", [KVOLD, DL], F32R, kind="ExternalInput").ap()
    tri = nc.dram_tensor("tri", [128, 128], F32, kind="ExternalInput").ap()
    ones_r_d = nc.dram_tensor("ones_r_d", [1, 512], F32R, kind="ExternalInput").ap()
    ones_c_d = nc.dram_tensor("ones_c_d", [128, 1], F32R, kind="ExternalInput").ap()

    ktn = nc.dram_tensor("ktn", [DL, QL], F32R, kind="ExternalOutput").ap()
    vn = nc.dram_tensor("vn", [QL, DL], F32R, kind="ExternalOutput").ap()
    outp = nc.dram_tensor("outp", [QL, D], F32, kind="ExternalOutput").ap()

    AF = mybir.ActivationFunctionType

    with TileContext(nc) as tc:
        with (
            tc.tile_pool(name="const", bufs=1) as cp,
            tc.tile_pool(name="qkv", bufs=8) as qkvp,
        ):
            ones_row = cp.tile([1, 512], F32R, name="ones_row", tag="ones_row")
            nc.sync.dma_start(ones_row[:], ones_r_d[:, :])
            ones_col = cp.tile([128, 1], F32R, name="ones_col", tag="ones_col")
            nc.sync.dma_start(ones_col[:], ones_c_d[:, :])
            tri_sb = cp.tile([128, 128], F32, name="tri_sb", tag="tri_sb")
            nc.sync.dma_start(tri_sb[:], tri[:, :])

            qT = [qkvp.tile([128, QL], F32R, name=f"qT{m}", tag="qT") for m in range(8)]
            kT = [qkvp.tile([128, QL], F32R, name=f"kT{m}", tag="kT") for m in range(8)]
            vv = [qkvp.tile([128, DL], F32R, name=f"vv{s}", tag="vv") for s in range(8)]

            # ---------------- phase A: projections ----------------
            with (
                tc.tile_pool(name="xp", bufs=16) as xp,
                tc.tile_pool(name="wp", bufs=24) as wp,
                tc.tile_pool(name="evA", bufs=4) as evp,
                tc.tile_pool(name="psA", bufs=4, space="PSUM") as pa,
            ):
                bq_sb = evp.tile([1, DL], F32R, name="bq_sb", tag="bq_sb", bufs=1)
                nc.sync.dma_start(bq_sb[:], bq_d[:, :])
                bv_sb = evp.tile([1, DL], F32R, name="bv_sb", tag="bv_sb", bufs=1)
                nc.sync.dma_start(bv_sb[:], bv_d[:, :])
                xT_t = []
                for dt in range(NDT):
                    t = xp.tile([128, QL], F32R, name=f"xTt{dt}", tag="xT")
                    nc.sync.dma_start(t[:], xT[dt * 128:(dt + 1) * 128, :])
                    xT_t.append(t)

                # qT and kT: out[dout_tile, seq] = W.T @ xT  (lhsT = W)
                for w_ap, bias_sb, out_arr, out_dram in (
                    (wq, bq_sb, qT, None),
                    (wk, None, kT, ktn),
                ):
                    for wc in range(4):
                        wt = []
                        for dt in range(NDT):
                            t = wp.tile([128, 256], F32R, name=f"w{wc}_{dt}", tag="w")
                            nc.sync.dma_start(
                                t[:], w_ap[dt * 128:(dt + 1) * 128,
                                           wc * 256:(wc + 1) * 256])
                            wt.append(t)
                        for mi in range(2):
                            m = wc * 2 + mi
                            for qh in range(2):
                                ps = pa.tile([128, 512], F32, name=f"psA{m}{qh}",
                                             tag="psA")
                                for dt in range(NDT):
                                    nc.tensor.matmul(
                                        ps[:],
                                        wt[dt][:, mi * 128:(mi + 1) * 128],
                                        xT_t[dt][:, qh * 512:(qh + 1) * 512],
                                        start=(dt == 0),
                                        stop=(dt == NDT - 1 and bias_sb is None),
                                    )
                                if bias_sb is not None:
                                    nc.tensor.matmul(
                                        ps[:],
                                        bias_sb[:, m * 128:(m + 1) * 128],
                                        ones_row[:, :512],
                                        start=False, stop=True,
                                    )
                                nc.vector.tensor_copy(
                                    out_arr[m][:, qh * 512:(qh + 1) * 512], ps[:])
                    if out_dram is not None:
                        for m in range(8):
                            nc.sync.dma_start(
                                out_dram[m * 128:(m + 1) * 128, :], out_arr[m][:])

                # v: out[seq_tile, dout] = xT.T @ Wv  (lhsT = xT)
                for wc in range(4):
                    wt = []
                    for dt in range(NDT):
                        t = wp.tile([128, 256], F32R, name=f"wv{wc}_{dt}", tag="w")
                        nc.sync.dma_start(
                            t[:], wv_w[dt * 128:(dt + 1) * 128,
                                       wc * 256:(wc + 1) * 256])
                        wt.append(t)
                    for s in range(8):
                        ps = pa.tile([128, 256], F32, name=f"psV{s}", tag="psA")
                        for dt in range(NDT):
                            nc.tensor.matmul(
                                ps[:],
                                xT_t[dt][:, s * 128:(s + 1) * 128],
                                wt[dt][:],
                                start=(dt == 0), stop=False,
                            )
                        nc.tensor.matmul(
                            ps[:],
                            ones_row[:, :128],
                            bv_sb[:, wc * 256:(wc + 1) * 256],
                            start=False, stop=True,
                        )
                        nc.vector.tensor_copy(
                            vv[s][:, wc * 256:(wc + 1) * 256], ps[:])
                for s in range(8):
                    nc.sync.dma_start(vn[s * 128:(s + 1) * 128, :], vv[s][:])

            # ---------------- phase B: attention ----------------
            with tc.tile_pool(name="wvTp", bufs=8) as wvtp:
                wvT = [wvtp.tile([128, QL], F32R, name=f"wvT{h}", tag="wvT")
                       for h in range(HL)]
                with (
                    tc.tile_pool(name="kchp", bufs=2) as kchp,
                    tc.tile_pool(name="vchp", bufs=2) as vchp,
                    tc.tile_pool(name="expp", bufs=3) as xpp,
                    tc.tile_pool(name="dsp", bufs=3) as dsp,
                    tc.tile_pool(name="psB", bufs=1, space="PSUM") as pb,
                ):
                    # vc_old rows for head h as [128, (t c)] SBUF tile: partition
                    # p, free (t, c) <- HBM row t*128+p, col h*128+c
                    vco_r = vc_old.rearrange("(t p) c -> p t c", p=128)

                    def emit_ln(h, den_sb, ln_sb):
                        nc.scalar.activation(ln_sb[:], den_sb[:], AF.Ln)

                    def emit_rec(h, ln_sb, rec_sb):
                        nc.scalar.activation(rec_sb[:], ln_sb[:], AF.Exp,
                                             scale=-1.0)

                    def emit_bc_mult(h, rec_sb):
                        for a in (0, 512):
                            aux = pb.tile([128, 512], F32, name=f"aux{h}_{a}",
                                          tag="aux", bufs=1)
                            nc.tensor.matmul(
                                aux[:], ones_row[:, :128],
                                rec_sb[:, a:a + 512], start=True, stop=True)
                            nc.vector.tensor_mul(
                                wvT[h][:, a:a + 512], wvT[h][:, a:a + 512],
                                aux[:])

                    pending = None
                    for h in range(HL):
                        kch = kchp.tile([128, KVOLD], F32R, name=f"kch{h}", tag="kch")
                        vch = vchp.tile([128, KVOLD], F32R, name=f"vch{h}", tag="vch")
                        for q4 in range(4):
                            cw = KVOLD // 4
                            sl = slice(q4 * cw, (q4 + 1) * cw)
                            nc.sync.dma_start(
                                kch[:, sl], kct_old[h * 128:(h + 1) * 128, sl])
                            nc.sync.dma_start(
                                vch[:, sl].rearrange("p (t c) -> p t c", c=128),
                                vco_r[:, q4 * (cw // 128):(q4 + 1) * (cw // 128),
                                      h * 128:(h + 1) * 128])

                        wv_ps = pb.tile([128, QL], F32, name=f"wvps{h}", tag="wv",
                                        bufs=1)
                        den_lo = pb.tile([1, 512], F32, name=f"denlo{h}",
                                         tag="den_lo", bufs=1)
                        den_hi = pb.tile([1, 512], F32, name=f"denhi{h}",
                                         tag="den_hi", bufs=1)

                        for kt in range(NKT):
                            # deferred normalize chain of the previous head,
                            # spread over this head's early iterations
                            if pending is not None:
                                ph, pden, pln, prec = pending
                                if kt == 3:
                                    emit_ln(ph, pden, pln)
                                elif kt == 5:
                                    emit_rec(ph, pln, prec)
                                elif kt == 7:
                                    emit_bc_mult(ph, prec)

                            j = kt - (NKT - NEWKT)  # >= 0 in the new-cache region
                            qs = 128 * j if (causal and j > 0) else 0
                            if j >= 0:
                                k_l = kT[h][:, j * 128:(j + 1) * 128]
                                v_l = vv[j][:, h * 128:(h + 1) * 128]
                            else:
                                k_l = kch[:, kt * 128:(kt + 1) * 128]
                                v_l = vch[:, kt * 128:(kt + 1) * 128]

                            segs = ([(qs, 512), (512, QL)] if qs < 512
                                    else [(qs, QL)])
                            et = xpp.tile([128, QL], F32R, name=f"et{h}_{kt}",
                                          tag="expp")
                            first = (kt == 0)
                            last_lo = (NKT - NEWKT + 3) if causal else (NKT - 1)
                            for si, (a, e) in enumerate(segs):
                                st = pb.tile([128, 512], F32, name=f"st{h}_{kt}_{a}",
                                             tag="sT", bufs=3)
                                nc.tensor.matmul(
                                    st[:, 0:e - a], k_l, qT[h][:, a:e],
                                    start=True, stop=True)
                                if si == 0 and causal and j >= 0:
                                    nc.vector.tensor_add(
                                        st[:, 0:128], st[:, 0:128], tri_sb[:])
                                nc.scalar.activation(
                                    et[:, a:e], st[:, 0:e - a], AF.Exp,
                                    scale=SCALE)
                                lo = a < 512
                                stop = (kt == (last_lo if lo else NKT - 1))
                                nc.tensor.matmul(
                                    wv_ps[:, a:e], v_l, et[:, a:e],
                                    start=first, stop=stop,
                                    skip_group_check=True)
                                if lo:
                                    d_out = den_lo[0:1, a:512]
                                else:
                                    d_out = den_hi[0:1, a - 512:e - 512]
                                nc.tensor.matmul(
                                    d_out, ones_col[:, :], et[:, a:e],
                                    start=first, stop=stop,
                                    skip_group_check=True)

                        # evacuate unnormalized wv and the denominators
                        nc.vector.tensor_copy(wvT[h][:], wv_ps[:])
                        den_sb = dsp.tile([1, QL], F32, name=f"densb{h}", tag="ds")
                        nc.vector.tensor_copy(den_sb[:, 0:512], den_lo[0:1, :])
                        nc.vector.tensor_copy(den_sb[:, 512:QL], den_hi[0:1, :])
                        ln_sb = dsp.tile([1, QL], F32, name=f"lnsb{h}", tag="ds")
                        rec_sb = dsp.tile([1, QL], F32R, name=f"recsb{h}",
                                          tag="ds")
                        pending = (h, den_sb, ln_sb, rec_sb)

                    # flush the last head's chain
                    ph, pden, pln, prec = pending
                    emit_ln(ph, pden, pln)
                    emit_rec(ph, pln, prec)
                    emit_bc_mult(ph, prec)

                # ---------------- phase C: out projection ----------------
                with (
                    tc.tile_pool(name="wop", bufs=16) as wop,
                    tc.tile_pool(name="ocp", bufs=4) as ocp,
                    tc.tile_pool(name="psC", bufs=4, space="PSUM") as pc,
                ):
                    for n in range(4):
                        wot = []
                        for dt in range(8):
                            t = wop.tile([128, 512], F32R, name=f"wo{n}_{dt}",
                                         tag="wo")
                            nc.sync.dma_start(
                                t[:], wo[dt * 128:(dt + 1) * 128,
                                         n * 512:(n + 1) * 512])
                            wot.append(t)
                        for m in range(8):
                            ps = pc.tile([128, 512], F32, name=f"psC{n}{m}",
                                         tag="psC")
                            for dt in range(8):
                                nc.tensor.matmul(
                                    ps[:],
                                    wvT[dt][:, m * 128:(m + 1) * 128],
                                    wot[dt][:],
                                    start=(dt == 0), stop=(dt == 7))
                            ot = ocp.tile([128, 512], F32, name=f"oc{n}{m}",
                                          tag="oc")
                            nc.vector.tensor_copy(ot[:], ps[:])
                            nc.sync.dma_start(
                                outp[m * 128:(m + 1) * 128,
                                     n * 512:(n + 1) * 512], ot[:])
    nc.finalize()
    return nc


_NC_CACHE: dict = {}


def _get_nc(causal: bool) -> bass.Bass:
    if causal not in _NC_CACHE:
        _NC_CACHE[causal] = build_nc(causal)
    return _NC_CACHE[causal]


def _check_mask(mask: np.ndarray) -> bool:
    """True -> the standard causal mask; False -> all-zeros (full attention)."""
    if not np.any(mask):
        return False
    i = np.arange(QL)[:, None]
    j = np.arange(KVL)[None, :]
    causal = np.where(j > (KVL - QL) + i, float(np.min(mask)), 0.0)
    if np.array_equal(mask, causal.astype(mask.dtype)) and np.min(mask) < -1e6:
        return True
    raise ValueError("kernel only supports the causal or empty mask")


def _run(inputs: dict, trace: bool = False):
    x = np.asarray(inputs["x"], dtype=np.float32)
    k_cache = np.asarray(inputs["k_cache"], dtype=np.float32)
    v_cache = np.asarray(inputs["v_cache"], dtype=np.float32)
    mask = np.asarray(inputs["mask"], dtype=np.float32)
    Wq = np.asarray(inputs["Wq"], dtype=np.float32)
    bq = np.asarray(inputs["bq"], dtype=np.float32)
    Wk = np.asarray(inputs["Wk"], dtype=np.float32)
    Wv = np.asarray(inputs["Wv"], dtype=np.float32)
    bv = np.asarray(inputs["bv"], dtype=np.float32)
    Wo = np.asarray(inputs["Wo"], dtype=np.float32)
    bo = np.asarray(inputs["bo"], dtype=np.float32)

    causal = _check_mask(mask)
    nc = _get_nc(causal)

    tri_np = np.where(
        np.arange(128)[None, :] < np.arange(128)[:, None], NEG, 0.0
    ).astype(np.float32)

    in_maps = []
    for c in range(N_CORES):
        b, g = c // 2, c % 2
        cs = slice(g * DL, (g + 1) * DL)
        in_maps.append({
            "xT": np.ascontiguousarray(x[b].T),
            "wq": np.ascontiguousarray(Wq[:, cs]),
            "wk": np.ascontiguousarray(Wk[:, cs]),
            "wv_w": np.ascontiguousarray(Wv[:, cs]),
            "bq_d": np.ascontiguousarray(bq[cs][None, :]),
            "bv_d": np.ascontiguousarray(bv[cs][None, :]),
            "wo": np.ascontiguousarray(Wo[cs, :]),
            "kct_old": np.ascontiguousarray(k_cache[b, :KVOLD, cs].T),
            "vc_old": np.ascontiguousarray(v_cache[b, :KVOLD, cs]),
            "tri": tri_np,
            "ones_r_d": np.ones((1, 512), np.float32),
            "ones_c_d": np.ones((128, 1), np.float32),
        })

    bkr = run_bass_kernel_spmd(nc, in_maps, list(range(N_CORES)), trace=trace)
    res = bkr.results

    kc = k_cache.copy()
    vc = v_cache.copy()
    out = np.empty((B, QL, D), dtype=np.float32)
    for c in range(N_CORES):
        b, g = c // 2, c % 2
        cs = slice(g * DL, (g + 1) * DL)
        kc[b, KVOLD:, cs] = res[c]["ktn"].T
        vc[b, KVOLD:, cs] = res[c]["vn"]
    for b in range(B):
        out[b] = res[2 * b]["outp"] + res[2 * b + 1]["outp"] + bo

    return (out, kc, vc), bkr


def kernel(**inputs):
    (out, kc, vc), _ = _run(inputs, trace=bool(os.environ.get("BASS_TRACE")))
    return out, kc, vc
